# revision 8
# baseline (speedup 1.0000x reference)
"""AdaptiveSpline forward on 8 TRN2 NeuronCores (Bass/Tile).

Math: the reference spline
    out(x) = sum_j coeffs[j] * prod_{i=1..3} clamp((x - t_j)/(t_{j+i} - t_j), 0, 1)
with uniform knots t_k = t0 + k*h is, on each knot interval, an exact cubic
polynomial.  Writing u = (x - t0)/h and s_k = clamp(u - k, 0, 1) it collapses to

    out = A0 + sum_{k=0}^{62} [ gam_k * s_k + bet_k * s_k^2 + alp_k * s_k^3 ]

with per-knot scalars derived from coeffs (bounded features -> good fp32
numerics).  Per knot the device computes (two engine recipes, mixed to balance
ScalarE/VectorE):

  recipe A (ScalarE-heavy):
    w = Relu(u - k)            [ScalarE, affine folded]
    v = Relu(1 - w)  (= 1-s)   [ScalarE]
    q = Square(1 - v) (= s^2)  [ScalarE]
    r = (v - c) * q            [VectorE scalar_tensor_tensor], c = (alp+bet)/alp
    psum += (-gam)*v + (-alp)*r     [TensorE scaled-identity matmuls, fp32r]

  recipe C (VectorE-heavy):
    t = clamp(x, X_k, X_{k+1})           [VectorE dual-op tensor_scalar]
    q = Square(t/h - X_k/h) (= s^2)      [ScalarE]
    r = (t - d) * q                      [VectorE], d = X_k - h*bet/alp
    psum += (gam/h)*t + (alp/h)*r        [TensorE]

The scaled identities live in SBUF (built once on GPSIMD); TensorE accumulates
all knot contributions into PSUM; a final ScalarE Identity(+const bias) evicts.

Sharding: pure data parallel - x split into 8 contiguous shards of 262144,
one per NeuronCore; knots/coeffs are folded into instruction immediates.
"""

import os
import numpy as np

N_TOTAL = 2_097_152
N_CORES = 8
P = 128
SHARD = N_TOTAL // N_CORES          # 262144
W = SHARD // P                      # 2048 fp32 per partition
BANK = 512                          # psum bank width (fp32)
NB = W // BANK                      # 4 banks

NUM_KNOTS = 64
DEG = 3
NI = NUM_KNOTS - 1                  # 63 intervals / knots in the s-basis

# fraction of knots on the VectorE-heavy recipe C (tunable; rest on recipe A)
LAMBDA_C = float(os.environ.get("KERNEL_LAMBDA_C", "0.5"))
ALPHA_EPS = 1e-7

_CACHE: dict = {}


def _tables(knots: np.ndarray, coeffs: np.ndarray):
    """Host-side (float64) per-knot scalars for the clamped-power basis."""
    kd = knots.astype(np.float64)
    cd = coeffs.astype(np.float64)
    K = NUM_KNOTS - 1 - DEG          # 60 basis functions
    h = (kd[-1] - kd[0]) / (NUM_KNOTS - 1)
    assert np.allclose(np.diff(kd), h, rtol=1e-4, atol=1e-6), "knots not uniform"
    t0 = kd[0]

    def c(j):
        return cd[j] if 0 <= j < K else 0.0

    def cum(k):                       # sum_{j<=k} c_j
        if k < 0:
            return 0.0
        return float(cd[: min(k, K - 1) + 1].sum())

    alp = np.zeros(NI)
    bet = np.zeros(NI)
    gam = np.zeros(NI)
    for m in range(NI):
        gam[m] = c(m - 2) / 3.0 + c(m - 1) / 3.0
        bet[m] = c(m - 1) / 6.0
        alp[m] = c(m) / 6.0
    a0 = cum(-3 + 0) + (2.0 / 3.0) * c(-2) + (1.0 / 6.0) * c(-1)  # == 0 terms
    a0 = (2.0 / 3.0) * c(-2) + (1.0 / 6.0) * c(-1)                # A[0]
    return h, t0, a0, alp, bet, gam


def _build(knots: np.ndarray, coeffs: np.ndarray):
    from concourse import bacc, mybir
    import concourse.tile as tile

    h, t0, a0, alp, bet, gam = _tables(knots, coeffs)
    su = 1.0 / h
    bu = -t0 / h

    # plan knots: recipe + matmul weights -----------------------------------
    # each entry: (kind, params...)
    plan = []          # per knot dict
    fills = []         # identity fill values, in emission order
    const = a0
    n_c = 0
    for k in range(NI):
        a_, b_, g_ = alp[k], bet[k], gam[k]
        if a_ == 0.0 and b_ == 0.0 and g_ == 0.0:
            continue
        use_c = (n_c + 1) / (len(plan) + 1) <= LAMBDA_C
        cub = abs(a_) > ALPHA_EPS * max(abs(b_), 1.0)
        ent = {"k": k, "recipe": "C" if use_c else "A", "cubic": cub}
        if use_c:
            n_c += 1
            Xk = t0 + k * h
            Xk1 = t0 + (k + 1) * h
            ent["clamp"] = (Xk, Xk1)
            if g_ != 0.0:
                ent["w_lin"] = len(fills)
                fills.append(g_ / h)
                const += -g_ * Xk / h
            ent["sq"] = (su, -Xk * su)          # Square(scale*t + bias) = s^2
            if cub:
                ent["stt_c"] = Xk - h * b_ / a_
                ent["w_cub"] = len(fills)
                fills.append(a_ / h)
            elif b_ != 0.0:
                ent["w_sq"] = len(fills)
                fills.append(b_)
        else:
            ent["relu_bias"] = bu - k
            if g_ != 0.0:
                ent["w_lin"] = len(fills)
                fills.append(-g_)
                const += g_
            ent["sq"] = (-1.0, 1.0)             # Square(1 - v) = s^2
            if cub:
                ent["stt_c"] = (a_ + b_) / a_
                ent["w_cub"] = len(fills)
                fills.append(-a_)
            elif b_ != 0.0:
                ent["w_sq"] = len(fills)
                fills.append(b_)
        if not any(key in ent for key in ("w_lin", "w_cub", "w_sq")):
            continue  # knot contributes nothing (e.g. k=62)
        plan.append(ent)
    nweights = len(fills)

    # build the Bass graph ---------------------------------------------------
    nc = bacc.Bacc("TRN2", target_bir_lowering=False, debug=False,
                   num_devices=N_CORES)
    f32 = mybir.dt.float32
    f32r = mybir.dt.float32r

    x_dram = nc.dram_tensor("x", [SHARD], f32, kind="ExternalInput")
    out_dram = nc.dram_tensor("out", [SHARD], f32, kind="ExternalOutput")
    x_2d = x_dram.ap().rearrange("(p w) -> p w", p=P)
    out_2d = out_dram.ap().rearrange("(p w) -> p w", p=P)

    # collect every ScalarE bias value used (activation bias must be an AP)
    bias_vals: list[float] = []

    def bias_idx(v: float) -> int:
        v = float(np.float32(v))
        if v not in bias_vals:
            bias_vals.append(v)
        return bias_vals.index(v)

    for ent in plan:
        if ent["recipe"] == "A":
            ent["relu_bias_i"] = bias_idx(ent["relu_bias"])
            ent["one_i"] = bias_idx(1.0)
        if "w_cub" in ent or "w_sq" in ent:
            ent["sq_bias_i"] = bias_idx(ent["sq"][1])
    const_i = bias_idx(const)

    with tile.TileContext(nc) as tc:
        with (
            tc.tile_pool(name="const", bufs=1) as cpool,
            tc.tile_pool(name="work", bufs=2) as work,
            tc.tile_pool(name="outp", bufs=1) as outp,
            tc.tile_pool(name="psum", bufs=1, space="PSUM") as psum,
        ):
            biases = cpool.tile([P, len(bias_vals)], f32, tag="biases")
            for bi, bv in enumerate(bias_vals):
                nc.gpsimd.memset(biases[:, bi:bi + 1], bv)
            # identity weights (one [P, 128] slice per matmul weight)
            ident = cpool.tile([P, nweights * P], f32r, tag="ident")
            zcol = cpool.tile([P, 1], f32, tag="zcol")
            nc.gpsimd.memset(zcol[:], 0.0)
            zbc = zcol[:].broadcast_to([P, P])
            for wi, val in enumerate(fills):
                nc.gpsimd.affine_select(
                    out=ident[:, wi * P:(wi + 1) * P],
                    in_=zbc,
                    compare_op=mybir.AluOpType.not_equal,
                    fill=float(val),
                    base=0,
                    pattern=[[-1, P]],
                    channel_multiplier=1,
                )

            x_tile = cpool.tile([P, W], f32, tag="x")
            nc.sync.dma_start(out=x_tile[:], in_=x_2d)

            acc = [psum.tile([P, BANK], f32, tag=f"ps{b}", name=f"ps{b}")
                   for b in range(NB)]
            started = [False] * NB
            n_mm = sum(1 for e in plan for key in ("w_lin", "w_cub", "w_sq")
                       if key in e)
            mm_done = [0] * NB

            def mm(b, wi, rhs):
                first = not started[b]
                started[b] = True
                mm_done[b] += 1
                nc.tensor.matmul(
                    acc[b][:],
                    ident[:, wi * P:(wi + 1) * P],
                    rhs[:, b * BANK:(b + 1) * BANK],
                    start=first,
                    stop=(mm_done[b] == n_mm),
                )

            for ent in plan:
                k = ent["k"]
                if ent["recipe"] == "C":
                    tt = work.tile([P, W], f32r, tag="t")
                    Xk, Xk1 = ent["clamp"]
                    nc.vector.tensor_scalar(
                        tt[:], x_tile[:], float(Xk), float(Xk1),
                        mybir.AluOpType.max, mybir.AluOpType.min)
                    lin = tt
                else:
                    wt = work.tile([P, W], f32, tag="w")
                    nc.scalar.activation(
                        wt[:], x_tile[:], mybir.ActivationFunctionType.Relu,
                        bias=biases[:, ent["relu_bias_i"]:ent["relu_bias_i"] + 1],
                        scale=float(su))
                    vt = work.tile([P, W], f32r, tag="v")
                    nc.scalar.activation(
                        vt[:], wt[:], mybir.ActivationFunctionType.Relu,
                        bias=biases[:, ent["one_i"]:ent["one_i"] + 1],
                        scale=-1.0)
                    lin = vt
                if "w_lin" in ent:
                    for b in range(NB):
                        mm(b, ent["w_lin"], lin)
                if "w_cub" in ent or "w_sq" in ent:
                    qt = work.tile([P, W], f32r, tag="q")
                    sc, bi = ent["sq"]
                    nc.scalar.activation(
                        qt[:], lin[:], mybir.ActivationFunctionType.Square,
                        bias=biases[:, ent["sq_bias_i"]:ent["sq_bias_i"] + 1],
                        scale=float(sc))
                    if "w_cub" in ent:
                        rt = work.tile([P, W], f32r, tag="r")
                        nc.vector.scalar_tensor_tensor(
                            rt[:], lin[:], float(ent["stt_c"]), qt[:],
                            mybir.AluOpType.subtract, mybir.AluOpType.mult)
                        for b in range(NB):
                            mm(b, ent["w_cub"], rt)
                    else:
                        for b in range(NB):
                            mm(b, ent["w_sq"], qt)

            out_tile = outp.tile([P, W], f32, tag="o")
            for b in range(NB):
                nc.scalar.activation(
                    out_tile[:, b * BANK:(b + 1) * BANK], acc[b][:],
                    mybir.ActivationFunctionType.Identity,
                    bias=biases[:, const_i:const_i + 1], scale=1.0)
            nc.sync.dma_start(out=out_2d, in_=out_tile[:])

    nc.compile()
    return nc


def _get_nc(knots: np.ndarray, coeffs: np.ndarray):
    key = (knots.astype(np.float32).tobytes(), coeffs.astype(np.float32).tobytes(),
           LAMBDA_C)
    if key not in _CACHE:
        _CACHE[key] = _build(knots, coeffs)
    return _CACHE[key]


LAST_RESULT = None


def _ensure_trace_hook() -> bool:
    """The image's antenv lacks axon_hooks; shim it so trace=True works."""
    try:
        from antenv.axon_hooks import get_axon_ntff_profile_hook  # noqa: F401
        return True
    except ImportError:
        pass
    try:
        import sys
        import types
        mod = types.ModuleType("antenv.axon_hooks")
        mod._hook = None

        def set_axon_ntff_profile_hook(h):
            mod._hook = h

        def get_axon_ntff_profile_hook():
            return mod._hook

        mod.set_axon_ntff_profile_hook = set_axon_ntff_profile_hook
        mod.get_axon_ntff_profile_hook = get_axon_ntff_profile_hook
        sys.modules["antenv.axon_hooks"] = mod
        import antenv
        antenv.axon_hooks = mod
        from trn_agent_boot.trn_boot import _ntff_profile_via_ctypes
        hook = _ntff_profile_via_ctypes("/opt/axon/libaxon_pjrt.so")
        mod._hook = hook
        return hook is not None
    except Exception:
        return False


def kernel(x: np.ndarray, knots: np.ndarray, coeffs: np.ndarray) -> np.ndarray:
    global LAST_RESULT
    from concourse.bass_utils import run_bass_kernel_spmd

    x = np.ascontiguousarray(np.asarray(x, dtype=np.float32))
    assert x.shape == (N_TOTAL,)
    nc = _get_nc(np.asarray(knots), np.asarray(coeffs))

    shards = x.reshape(N_CORES, SHARD)
    in_maps = [{"x": np.ascontiguousarray(shards[i])} for i in range(N_CORES)]
    trace = bool(int(os.environ.get("KERNEL_TRACE", "0")))
    if trace:
        trace = _ensure_trace_hook()
    res = run_bass_kernel_spmd(
        nc, in_maps, core_ids=list(range(N_CORES)), trace=trace)
    LAST_RESULT = res
    out = np.concatenate([res.results[i]["out"].reshape(-1)
                          for i in range(N_CORES)])
    return out.astype(np.float32, copy=False)
